# revision 1
# baseline (speedup 1.0000x reference)
"""DiffJPEG TRN2 Bass kernel, v2.

Data-parallel over batch (4 images/core on 8 cores). Color transforms run
on the host (linear pre/post processing, exact in f32); the device runs the
pure per-channel blockwise 2D DCT -> quantize/round -> dequant -> 2D IDCT.

Device pipeline per channel-unit (12 units = 4 images x 3 channels, each a
[512, 512] plane processed as 4 row-bands / 4 column-chunks of [128, 512]):
  stage1  PE       A = (Lb/8) @ x        (vertical 8-pt DCT, 4 matmuls)
  p1      Act/DVE  evict psum -> A fp16
  T1      DMA      at = chunk-transpose(A)  (XBAR dma_start_transpose, 1 op)
  stage3  PE       F' = Lb @ at          (horizontal DCT, 4 matmuls)
  p2      DVE      rq = int16(F' * 8/QT) (fused quantize + RNE round)
  p3      Pool/DVE dq = fp16(rq * QT)    (dequant, exact in fp16)
  stage5  PE       f = (Lb/8)^T @ dq     (horizontal IDCT, 4 matmuls)
  p4      Act/DVE  evict psum -> f fp16
  T2      PE       g = transpose(f)      (16 [128,128] transposes, fp16 psum)
  p5      DVE      evict psum fp16 -> g  (band-paired [128,1024], 2x rate)
  stage7  PE       y = Lb^T @ g          (vertical IDCT, 4 matmuls)
  p6      Act/DVE  evict psum -> staging fp16 (values = YCC255/8)
  out     DMA      1 dma per channel

Units run through a software-pipelined dataflow emission (all units advance
round-robin; the tile scheduler overlaps phases across units). Engine splits
are tuned so DVE/Act both sit near saturation with Pool taking half the
dequants.

Numerics: forward coefficients reach quantization with ~0.05 abs error
(fp16 input + fp16 stationaries + scale-folding so fp16 ulps stay small),
so ~0.3% of coefficients flip a rounding bin vs the f32 reference
(rel_l2 ~ 5e-3, tolerance 2e-2). rq (|q| <= 1030) is exact int16 via the
hardware's RNE float->int convert (matches jnp.round); dq = rq*QT <= 2047
is exact in fp16.
"""
import math
import numpy as np

_N_CORES = 8
_B = 32
_BPC = _B // _N_CORES
_H = _W = 512
_NB = _H // 128   # row bands per channel

_state = {}


def _dct8_f64():
    D = np.zeros((8, 8), dtype=np.float64)
    for u in range(8):
        au = 1.0 / math.sqrt(2.0) if u == 0 else 1.0
        for x in range(8):
            D[u, x] = au * 0.5 * math.cos((2 * x + 1) * u * math.pi / 16.0)
    return D


def _y_quant_table():
    t = np.array([[16, 11, 10, 16, 24, 40, 51, 61], [12, 12, 14, 19, 26, 58, 60, 55],
                  [14, 13, 16, 24, 40, 57, 69, 56], [14, 17, 22, 29, 51, 87, 80, 62],
                  [18, 22, 37, 56, 68, 109, 103, 77], [24, 35, 55, 64, 81, 104, 113, 92],
                  [49, 64, 78, 87, 103, 121, 120, 101], [72, 92, 95, 98, 112, 100, 103, 99]],
                 dtype=np.float64).T
    return t


def _c_quant_table():
    t = np.full((8, 8), 99, dtype=np.float64)
    t[:4, :4] = np.array([[17, 18, 24, 47], [18, 21, 26, 66], [24, 26, 56, 99],
                          [47, 66, 99, 99]], dtype=np.float64).T
    return t


def _host_constants():
    D = _dct8_f64()
    Lb = np.kron(np.eye(16), D)            # [128,128] block-diag 8-pt DCT

    lb1 = np.asarray((Lb / 8.0).T, dtype=np.float16)   # stage1 lhsT: out = (Lb/8) @ x
    lb3 = np.asarray(Lb.T, dtype=np.float16)           # stage3 lhsT: out = Lb @ at
    lb5 = np.asarray(Lb / 8.0, dtype=np.float16)       # stage5 lhsT: out = (Lb/8)^T @ dq
    lb7 = np.asarray(Lb, dtype=np.float16)             # stage7 lhsT: out = Lb^T @ g

    # quant tables in the [wfreq(p), (band, rfreq)(f)] layout:
    # v = p % 8, u = f % 8; value pattern QT[u, v]
    QT = np.stack([_y_quant_table(), _c_quant_table(), _c_quant_table()])
    u = (np.arange(_W) % 8)[None, :]
    v = (np.arange(128) % 8)[:, None]
    qti = np.zeros((3, 128, _W), dtype=np.float32)
    qtt = np.zeros((3, 128, _W), dtype=np.float16)
    for c in range(3):
        pat = QT[c][u, v]
        qti[c] = (8.0 / pat).astype(np.float32)
        qtt[c] = pat.astype(np.float16)

    ident = np.eye(128, dtype=np.float16)
    cf32 = qti.transpose(1, 0, 2).reshape(128, 3 * _W).copy()
    cf16 = np.concatenate(
        [qtt.transpose(1, 0, 2).reshape(128, 3 * _W),
         lb1, lb3, lb5, lb7, ident], axis=1).astype(np.float16)
    return dict(cf32=cf32, cf16=cf16)


def _build_program():
    import sys
    if "/opt/trn_rl_repo" not in sys.path:
        sys.path.insert(0, "/opt/trn_rl_repo")
    from contextlib import ExitStack
    import concourse.bacc as bacc
    import concourse.tile as tile
    from concourse import mybir
    from concourse.alu_op_type import AluOpType

    F32 = mybir.dt.float32
    F16 = mybir.dt.float16
    I16 = mybir.dt.int16

    nc = bacc.Bacc("TRN2", target_bir_lowering=False, debug=False,
                   num_devices=_N_CORES)

    # ycc input: [img, ch, band, 128, 512] fp16 (host-mixed YCbCr*255 - off)
    x = nc.declare_dram_parameter("x", [_BPC, 3, _NB, 128, _W], F16,
                                  isOutput=False)
    # packed constants: cf32 = qti [128, 1536]; cf16 = qtt|lb1|lb3|lb5|lb7|ident
    cf32 = nc.declare_dram_parameter("cf32", [128, 3 * _W], F32, isOutput=False)
    cf16 = nc.declare_dram_parameter("cf16", [128, 3 * _W + 5 * 128], F16,
                                     isOutput=False)
    # out: [img, ch, band, 128, 512] fp16 (YCC255/8, unclipped)
    out = nc.declare_dram_parameter("out", [_BPC, 3, _NB, 128, _W], F16,
                                    isOutput=True)

    with tile.TileContext(nc) as tc, ExitStack() as ctx:
        cpool = ctx.enter_context(tc.tile_pool(name="consts", bufs=1))
        xpool = ctx.enter_context(tc.tile_pool(name="xp", bufs=8))
        apool = ctx.enter_context(tc.tile_pool(name="ap", bufs=6))
        atpool = ctx.enter_context(tc.tile_pool(name="atp", bufs=7))
        rqpool = ctx.enter_context(tc.tile_pool(name="rqp", bufs=12))
        dqpool = ctx.enter_context(tc.tile_pool(name="dqp", bufs=12))
        fpool = ctx.enter_context(tc.tile_pool(name="fp", bufs=7))
        gpool = ctx.enter_context(tc.tile_pool(name="gp", bufs=10))
        opool = ctx.enter_context(tc.tile_pool(name="op", bufs=6))
        ps1 = ctx.enter_context(tc.tile_pool(name="ps1", bufs=2, space="PSUM"))
        ps3 = ctx.enter_context(tc.tile_pool(name="ps3", bufs=2, space="PSUM"))
        ps5 = ctx.enter_context(tc.tile_pool(name="ps5", bufs=1, space="PSUM"))
        psT = ctx.enter_context(tc.tile_pool(name="psT", bufs=1, space="PSUM"))
        ps7 = ctx.enter_context(tc.tile_pool(name="ps7", bufs=2, space="PSUM"))

        t32 = cpool.tile([128, 3, _W], F32, tag="c_f32")
        nc.sync.dma_start(t32[:], cf32[:])
        t16 = cpool.tile([128, 3 * _W + 5 * 128], F16, tag="c_f16")
        nc.sync.dma_start(t16[:], cf16[:])
        ct = {"qti": t32}
        ct["qtt"] = t16[:, 0:3 * _W]
        for k, name in enumerate(("lb1", "lb3", "lb5", "lb7", "ident")):
            o = 3 * _W + k * 128
            ct[name] = t16[:, o:o + 128]

        st = {}  # per-unit tile handles; unit u = img * 3 + ci

        def load_unit(u):
            img, ci = divmod(u, 3)
            t = xpool.tile([128, _NB, _W], F16, tag="x")
            xr = x[img, ci].rearrange("b p w -> p b w")
            nc.sync.dma_start(t[:, 0:2, :], xr[:, 0:2, :])
            nc.sync.dma_start(t[:, 2:4, :], xr[:, 2:4, :])
            st[u] = {"xt": t}

        def phase1(u):
            # stage1 + p1 + T1-dmat for one channel
            A = apool.tile([128, _NB, _W], F16, tag="A")
            for b in range(_NB):
                p = ps1.tile([128, _W], F32, tag="s1")
                nc.tensor.matmul(p[:], ct["lb1"], st[u]["xt"][:, b, :],
                                 start=True, stop=True)
                if u < 2:
                    nc.vector.tensor_copy(A[:, b, :], p[:])
                else:
                    nc.scalar.copy(A[:, b, :], p[:])
                yield
            t = atpool.tile([128, 4 * _NB, 128], F16, tag="at")
            nc.sync.dma_start_transpose(t[:], A[:])
            st[u]["at"] = t

        def phase2(u):
            # stage3 + quant + dequant + stage5 + p4 for one channel
            img, ci = divmod(u, 3)
            at = st[u]["at"]
            f = fpool.tile([128, 4, _W], F16, tag="f")
            st[u]["f"] = f
            for wc in range(4):
                p = ps3.tile([128, _W], F32, tag="s3")
                nc.tensor.matmul(p[:], ct["lb3"], at[:, wc::4, :],
                                 start=True, stop=True)
                rq = rqpool.tile([128, _W], I16, tag="rq")
                nc.vector.tensor_tensor(rq[:], p[:], ct["qti"][:, ci, :],
                                        op=AluOpType.mult)
                dq = dqpool.tile([128, _W], F16, tag="dq")
                eng = nc.gpsimd if wc < 2 else nc.vector
                eng.tensor_tensor(dq[:], rq[:],
                                  ct["qtt"][:, ci * _W:(ci + 1) * _W],
                                  op=AluOpType.mult)
                p5t = ps5.tile([128, _W], F32, tag="s5")
                nc.tensor.matmul(p5t[:], ct["lb5"], dq[:], start=True,
                                 stop=True)
                if wc < 3:
                    nc.scalar.copy(f[:, wc, :], p5t[:])
                else:
                    nc.vector.tensor_copy(f[:, wc, :], p5t[:])
                yield

        def phase3(u):
            # T2 (paired bands) + p5 + stage7 + p6 + per-channel out DMA
            img, ci = divmod(u, 3)
            f = st[u]["f"]
            ot = opool.tile([128, _NB, _W], F16, tag="o")
            for pair in range(2):
                pg = psT.tile([128, 2, _W], F16, tag="tps")
                for k in range(2):
                    b = pair * 2 + k
                    for wc in range(4):
                        nc.tensor.transpose(
                            pg[:, k, wc * 128:(wc + 1) * 128],
                            f[:, wc, b * 128:(b + 1) * 128], ct["ident"])
                g = gpool.tile([128, 2, _W], F16, tag="g")
                nc.vector.tensor_copy(g[:], pg[:])
                yield
                for k in range(2):
                    b = pair * 2 + k
                    p7 = ps7.tile([128, _W], F32, tag="s7")
                    nc.tensor.matmul(p7[:], ct["lb7"], g[:, k, :],
                                     start=True, stop=True)
                    nc.scalar.copy(ot[:, b, :], p7[:])
                    yield
            orr = out[img, ci].rearrange("b p w -> p b w")
            for bb in range(4):
                nc.sync.dma_start(orr[:, bb, :], ot[:, bb, :])
            del st[u]["f"]

        # 3-deep software pipeline over channel units.
        U = 3 * _BPC
        for u in range(min(3, U)):
            load_unit(u)
        # global dataflow emission: all units progress round-robin; a unit's
        # next phase activates when the previous one finishes emitting.
        active = {u: None for u in range(U)}
        stage = {u: 0 for u in range(U)}
        loaded = 3
        ndone = 0
        while ndone < U:
            for u in range(U):
                if stage[u] >= 3:
                    continue
                if active[u] is None:
                    if stage[u] == 0 and u < loaded:
                        active[u] = phase1(u)
                    elif stage[u] == 1:
                        if loaded < U:
                            load_unit(loaded)
                            loaded += 1
                        active[u] = phase2(u)
                    elif stage[u] == 2:
                        active[u] = phase3(u)
                    else:
                        continue
                try:
                    next(active[u])
                except StopIteration:
                    active[u] = None
                    stage[u] += 1
                    if stage[u] == 3:
                        ndone += 1

    nc.compile()
    return nc, _host_constants()


def _get_program():
    if "nc" not in _state:
        _state["nc"] = _build_program()
    return _state["nc"]


def _host_forward(image):
    """clip + RGB->YCbCr(255, offset) in f32, exactly as the reference."""
    x = np.clip(image.astype(np.float32, copy=False), 0.0, 1.0)
    r, g, b = x[:, 0], x[:, 1], x[:, 2]
    y = 0.299 * r + 0.587 * g + 0.114 * b
    cb = (b - y) * np.float32(0.564) + np.float32(0.5)
    cr = (r - y) * np.float32(0.713) + np.float32(0.5)
    ycc = np.stack([y, cb, cr], axis=1)
    return (ycc * np.float32(255.0) - np.float32(128.0)).astype(np.float16)


def _host_inverse(yout):
    """yout: [B,3,H,W] fp16 = YCC255/8 (offset domain). Returns f32 RGB."""
    v = yout.astype(np.float32) * np.float32(8.0)
    px = (v + np.float32(128.0)) / np.float32(255.0)
    yy = px[:, 0]
    cb = px[:, 1] - np.float32(0.5)
    cr = px[:, 2] - np.float32(0.5)
    r = yy + np.float32(1.403) * cr
    g = yy - np.float32(0.714) * cr - np.float32(0.344) * cb
    b = yy + np.float32(1.773) * cb
    rgb = np.stack([r, g, b], axis=1)
    return np.clip(rgb, 0.0, 1.0).astype(np.float32)


def kernel(image: np.ndarray) -> np.ndarray:
    import sys
    if "/opt/trn_rl_repo" not in sys.path:
        sys.path.insert(0, "/opt/trn_rl_repo")
    from concourse.bass_utils import run_bass_kernel_spmd

    image = np.asarray(image)
    assert image.shape == (_B, 3, _H, _W), image.shape
    nc, consts = _get_program()

    ycc = _host_forward(image)                        # [32,3,512,512] fp16
    ycc = ycc.reshape(_B, 3, _NB, 128, _W)

    in_maps = []
    for c in range(_N_CORES):
        sl = slice(c * _BPC, (c + 1) * _BPC)
        m = dict(x=ycc[sl])
        m.update(consts)
        in_maps.append(m)

    res = run_bass_kernel_spmd(nc, in_maps, core_ids=list(range(_N_CORES)))
    _state["exec_time_ns"] = getattr(res, "exec_time_ns", None)
    outs = [res.results[c]["out"] for c in range(_N_CORES)]
    yfull = np.concatenate(outs, axis=0).reshape(_B, 3, _H, _W)
    return _host_inverse(yfull)


if __name__ == "__main__":
    rng = np.random.default_rng(0)
    img = rng.uniform(size=(_B, 3, _H, _W)).astype(np.float32)
    o = kernel(img)
    print(o.shape, o.dtype, float(o.min()), float(o.max()))



# revision 2
# speedup vs baseline: 2.7347x; 2.7347x over previous
"""DiffJPEG TRN2 Bass kernel, v3 — block-column (kron) dataflow.

Data-parallel over batch (4 images/core on 8 cores). The host does the
linear color transforms plus a free block-flattening reshape: each 8x8
image block becomes a 64-element column, two blocks stacked per SBUF
partition. In that layout the whole 2D DCT is ONE block-diagonal matmul
(kron(I2, kron(D8, D8))), the quant table varies only along the partition
axis, and no transposes are needed anywhere.

Device pipeline per plane (12 planes = 4 images x 3 channels, each
[128 part = 2x64 block positions, 2048 free = block pairs], fp16):
  mmF   PE   F = (M2/8) @ x64          -> PSUM f32   (4 matmuls, 512 free)
  quant A/D  rq = f16(F*(8/Q[p]) + 1536)             (RNE on the fp16
             integer grid since 1024 <= rq < 2048; |q| <= ~110)
  mmI   PE   P = (M2^T diag(Q)/8) @ rq -> PSUM f32   (dequant folded into
             the per-channel stationary; +1536 offset is linear)
  evict A/D  u8 = sat(RNE(8*P + bias[p]))            (bias cancels the
             1536 offset and adds +128; output = YCC255 pixels, uint8)
Quant/evict ops alternate between Activation and DVE by a greedy
cost-balancer; Pool cannot access PSUM so it idles. Output DMA is uint8
(half the bytes of fp16); the +-0.5/255 YCC rounding adds ~2e-3 rel err.

Per-core cost model: DMA ~26us (in 12x1456 + out 12x728), PE ~21us warm
(96 matmuls x 512 cols), Act ~25us, DVE ~27us.
"""
import math
import numpy as np

_N_CORES = 8
_B = 32
_BPC = _B // _N_CORES
_H = _W = 512

_state = {}


def _dct8_f64():
    D = np.zeros((8, 8), dtype=np.float64)
    for u in range(8):
        au = 1.0 / math.sqrt(2.0) if u == 0 else 1.0
        for x in range(8):
            D[u, x] = au * 0.5 * math.cos((2 * x + 1) * u * math.pi / 16.0)
    return D


def _y_quant_table():
    t = np.array([[16, 11, 10, 16, 24, 40, 51, 61], [12, 12, 14, 19, 26, 58, 60, 55],
                  [14, 13, 16, 24, 40, 57, 69, 56], [14, 17, 22, 29, 51, 87, 80, 62],
                  [18, 22, 37, 56, 68, 109, 103, 77], [24, 35, 55, 64, 81, 104, 113, 92],
                  [49, 64, 78, 87, 103, 121, 120, 101], [72, 92, 95, 98, 112, 100, 103, 99]],
                 dtype=np.float64).T
    return t


def _c_quant_table():
    t = np.full((8, 8), 99, dtype=np.float64)
    t[:4, :4] = np.array([[17, 18, 24, 47], [18, 21, 26, 66], [24, 26, 56, 99],
                          [47, 66, 99, 99]], dtype=np.float64).T
    return t


_QOFF = 1536.0  # fp16 integer-grid rounding offset (ulp == 1 in [1024, 2048))


def _host_constants():
    D = _dct8_f64()
    M64 = np.kron(D, D)                       # [64,64] 2D DCT, orthonormal
    M2 = np.kron(np.eye(2), M64)              # [128,128] two blocks/partition

    # quant divisors along the partition axis: p%64 = 8u+v
    QT = np.stack([_y_quant_table(), _c_quant_table()])  # [2,8,8] (Y, C)
    p = np.arange(128)
    u, v = (p % 64) // 8, p % 8
    qvec = QT[:, u, v]                        # [2,128] divisor per partition

    mmF = np.asarray((M2 / 8.0).T, dtype=np.float16)        # fwd stationary
    mmI = [np.asarray(qvec[c][:, None] * M2 / 8.0, dtype=np.float16)
           for c in range(2)]                 # lhsT_i[k,p] = Q[k]*M2[k,p]/8

    # qs: quant scale per partition/channel-class; bias cancels the +_QOFF
    # offset after it flows through the (fp16-rounded) inverse stationary.
    qs = np.zeros((128, 2), dtype=np.float32)
    bias = np.zeros((128, 2), dtype=np.float32)
    for c in range(2):
        qs[:, c] = (8.0 / qvec[c]).astype(np.float32)
        colsum = mmI[c].astype(np.float64).sum(axis=0)      # [128]
        bias[:, c] = (128.0 - 8.0 * _QOFF * colsum).astype(np.float32)

    cmm = np.concatenate([mmF, mmI[0], mmI[1]], axis=1)     # [128, 384] f16
    csc = np.concatenate([qs, bias], axis=1)                # [128, 4] f32
    return dict(cmm=cmm, csc=csc)


def _build_program():
    import sys
    if "/opt/trn_rl_repo" not in sys.path:
        sys.path.insert(0, "/opt/trn_rl_repo")
    from contextlib import ExitStack
    import concourse.bacc as bacc
    import concourse.tile as tile
    from concourse import mybir
    from concourse.alu_op_type import AluOpType

    F32 = mybir.dt.float32
    F16 = mybir.dt.float16
    U8 = mybir.dt.uint8

    nc = bacc.Bacc("TRN2", target_bir_lowering=False, debug=False,
                   num_devices=_N_CORES)

    # x: block-flattened YCC255-128 fp16 (host-prepped)
    x = nc.declare_dram_parameter("x", [_BPC, 3, 128, 2048], F16,
                                  isOutput=False)
    cmm = nc.declare_dram_parameter("cmm", [128, 384], F16, isOutput=False)
    csc = nc.declare_dram_parameter("csc", [128, 4], F32, isOutput=False)
    # out: YCC255 pixels, uint8, same block-flattened layout
    out = nc.declare_dram_parameter("out", [_BPC, 3, 128, 2048], U8,
                                    isOutput=True)

    U = 3 * _BPC  # 12 planes per core

    with tile.TileContext(nc) as tc, ExitStack() as ctx:
        cpool = ctx.enter_context(tc.tile_pool(name="consts", bufs=1))
        xpool = ctx.enter_context(tc.tile_pool(name="xp", bufs=4))
        rqpool = ctx.enter_context(tc.tile_pool(name="rqp", bufs=3))
        opool = ctx.enter_context(tc.tile_pool(name="op", bufs=3))
        psF = ctx.enter_context(tc.tile_pool(name="psF", bufs=2, space="PSUM"))
        psI = ctx.enter_context(tc.tile_pool(name="psI", bufs=2, space="PSUM"))

        t16 = cpool.tile([128, 384], F16, tag="c_f16")
        nc.sync.dma_start(t16[:], cmm[:])
        t32 = cpool.tile([128, 4], F32, tag="c_f32")
        nc.sync.dma_start(t32[:], csc[:])
        mmF = t16[:, 0:128]
        mmI = (t16[:, 128:256], t16[:, 256:384])  # Y, C stationaries

        st = {}
        # greedy Act/DVE balancer for the pointwise ops ([128,1024] each)
        bal = {"act": 0.0, "dve": 0.0}

        def pick_engine():
            if bal["act"] + 1038.0 <= bal["dve"] + 1192.0:
                bal["act"] += 1038.0
                return "act"
            bal["dve"] += 1192.0
            return "dve"

        def load_plane(u):
            img, ci = divmod(u, 3)
            t = xpool.tile([128, 2048], F16, tag="x")
            nc.sync.dma_start(t[:], x[img, ci])
            st[u] = {"xt": t}

        def phaseA(u):
            # forward DCT + quantize (fp16 integer-grid RNE via +_QOFF)
            img, ci = divmod(u, 3)
            cc = 0 if ci == 0 else 1
            qs = t32[:, cc:cc + 1]
            xt = st[u]["xt"]
            rq = rqpool.tile([128, 2048], F16, tag="rq")
            st[u]["rq"] = rq
            for h in range(2):
                pf = psF.tile([128, 2, 512], F32, tag="pf")
                for j in range(2):
                    k = 2 * h + j
                    nc.tensor.matmul(pf[:, j, :], mmF,
                                     xt[:, k * 512:(k + 1) * 512],
                                     start=True, stop=True)
                dst = rq[:, h * 1024:(h + 1) * 1024]
                if pick_engine() == "act":
                    nc.scalar.activation(dst, pf[:, :, :],
                                         mybir.ActivationFunctionType.Copy,
                                         bias=_QOFF, scale=qs)
                else:
                    nc.vector.tensor_scalar(dst, pf[:, :, :], qs, _QOFF,
                                            op0=AluOpType.mult,
                                            op1=AluOpType.add)
                yield

        def phaseB(u):
            # inverse DCT (dequant folded) + biased uint8 eviction
            img, ci = divmod(u, 3)
            cc = 0 if ci == 0 else 1
            bias = t32[:, 2 + cc:3 + cc]
            rq = st[u]["rq"]
            ot = opool.tile([128, 2048], U8, tag="ot")
            for h in range(2):
                pi = psI.tile([128, 2, 512], F32, tag="pi")
                for j in range(2):
                    k = 2 * h + j
                    nc.tensor.matmul(pi[:, j, :], mmI[cc],
                                     rq[:, k * 512:(k + 1) * 512],
                                     start=True, stop=True)
                dst = ot[:, h * 1024:(h + 1) * 1024]
                if pick_engine() == "act":
                    nc.scalar.activation(dst, pi[:, :, :],
                                         mybir.ActivationFunctionType.Identity,
                                         bias=bias, scale=8.0)
                else:
                    nc.vector.tensor_scalar(dst, pi[:, :, :], 8.0, bias,
                                            op0=AluOpType.mult,
                                            op1=AluOpType.add)
                yield
            nc.sync.dma_start(out[img, ci], ot[:])
            del st[u]["rq"], st[u]["xt"]

        # software pipeline: A(u) runs alongside B(u-1); loads 2 ahead
        load_plane(0)
        load_plane(1)
        prev_b = None
        for u in range(U):
            if u + 2 < U:
                load_plane(u + 2)
            a = phaseA(u)
            gens = [g for g in (a, prev_b) if g is not None]
            while gens:
                for g in list(gens):
                    try:
                        next(g)
                    except StopIteration:
                        gens.remove(g)
            prev_b = phaseB(u)
        for _ in prev_b:
            pass

    nc.compile()
    return nc, _host_constants()


def _get_program():
    if "nc" not in _state:
        _state["nc"] = _build_program()
    return _state["nc"]


def _host_forward(image):
    """clip + RGB->YCbCr(255, -128) f32, then block-flatten to fp16.

    Layout: partition p = 64*s + 8*y + xx (s = block-row parity, y/xx =
    row/col within the 8x8 block), free j = (block_row//2)*64 + block_col.
    """
    x = np.clip(image.astype(np.float32, copy=False), 0.0, 1.0)
    r, g, b = x[:, 0], x[:, 1], x[:, 2]
    y = np.float32(0.299) * r + np.float32(0.587) * g + np.float32(0.114) * b
    cb = (b - y) * np.float32(0.564) + np.float32(0.5)
    cr = (r - y) * np.float32(0.713) + np.float32(0.5)
    ycc = np.stack([y, cb, cr], axis=1)
    v = ycc * np.float32(255.0) - np.float32(128.0)
    B = v.shape[0]
    t = v.reshape(B, 3, 32, 2, 8, 64, 8)          # [B,3,brh,s,y,bc,xx]
    x64 = t.transpose(0, 1, 3, 4, 6, 2, 5)        # [B,3,s,y,xx,brh,bc]
    return np.ascontiguousarray(x64.reshape(B, 3, 128, 2048),
                                dtype=np.float16)


def _host_inverse(y64u8):
    """y64u8: [B,3,128,2048] uint8 YCC255 block-flattened -> f32 RGB."""
    B = y64u8.shape[0]
    w = y64u8.reshape(B, 3, 2, 8, 8, 32, 64)      # [B,3,s,y,xx,brh,bc]
    v = w.transpose(0, 1, 5, 2, 3, 6, 4)          # [B,3,brh,s,y,bc,xx]
    px = v.reshape(B, 3, _H, _W).astype(np.float32) / np.float32(255.0)
    yy = px[:, 0]
    cb = px[:, 1] - np.float32(0.5)
    cr = px[:, 2] - np.float32(0.5)
    r = yy + np.float32(1.403) * cr
    g = yy - np.float32(0.714) * cr - np.float32(0.344) * cb
    b = yy + np.float32(1.773) * cb
    rgb = np.stack([r, g, b], axis=1)
    return np.clip(rgb, 0.0, 1.0).astype(np.float32)


def kernel(image: np.ndarray) -> np.ndarray:
    import sys
    if "/opt/trn_rl_repo" not in sys.path:
        sys.path.insert(0, "/opt/trn_rl_repo")
    from concourse.bass_utils import run_bass_kernel_spmd

    image = np.asarray(image)
    assert image.shape == (_B, 3, _H, _W), image.shape
    nc, consts = _get_program()

    x64 = _host_forward(image)                    # [32,3,128,2048] f16
    x64 = x64.reshape(_B, 3, 128, 2048)

    in_maps = []
    for c in range(_N_CORES):
        sl = slice(c * _BPC, (c + 1) * _BPC)
        m = dict(x=x64[sl])
        m.update(consts)
        in_maps.append(m)

    res = run_bass_kernel_spmd(nc, in_maps, core_ids=list(range(_N_CORES)))
    _state["exec_time_ns"] = getattr(res, "exec_time_ns", None)
    outs = [res.results[c]["out"] for c in range(_N_CORES)]
    yfull = np.concatenate(outs, axis=0)
    return _host_inverse(yfull)


if __name__ == "__main__":
    rng = np.random.default_rng(0)
    img = rng.uniform(size=(_B, 3, _H, _W)).astype(np.float32)
    o = kernel(img)
    print(o.shape, o.dtype, float(o.min()), float(o.max()))


# revision 25
# speedup vs baseline: 2.9310x; 1.0718x over previous
"""DiffJPEG TRN2 Bass kernel, v3 — block-column (kron) dataflow.

Data-parallel over batch (4 images/core on 8 cores). The host does the
linear color transforms plus a free block-flattening reshape: each 8x8
image block becomes a 64-element column, two blocks stacked per SBUF
partition. In that layout the whole 2D DCT is ONE block-diagonal matmul
(kron(I2, kron(D8, D8))), the quant table varies only along the partition
axis, and no transposes are needed anywhere.

Device pipeline per plane (12 planes = 4 images x 3 channels, each
[128 part = 2x64 block positions, 2048 free = block pairs], fp16):
  mmF   PE   F = (M2/8) @ x64          -> PSUM f32   (4 matmuls, 512 free)
  quant A/D  rq = f16(F*(8/Q[p]) + 1536)             (RNE on the fp16
             integer grid since 1024 <= rq < 2048; |q| <= ~110)
  mmI   PE   P = (M2^T diag(Q)/8) @ rq -> PSUM f32   (dequant folded into
             the per-channel stationary; +1536 offset is linear)
  evict A/D  u8 = sat(RNE(8*P + bias[p]))            (bias cancels the
             1536 offset and adds +128; output = YCC255 pixels, uint8)
Quant/evict ops alternate between Activation and DVE by a greedy
cost-balancer; Pool cannot access PSUM so it idles. Output DMA is uint8
(half the bytes of fp16); the +-0.5/255 YCC rounding adds ~2e-3 rel err.

Per-core cost model: DMA ~26us (in 12x1456 + out 12x728), PE ~21us warm
(96 matmuls x 512 cols), Act ~25us, DVE ~27us.
"""
import math
import numpy as np

_N_CORES = 8
_B = 32
_BPC = _B // _N_CORES
_H = _W = 512

_state = {}


def _dct8_f64():
    D = np.zeros((8, 8), dtype=np.float64)
    for u in range(8):
        au = 1.0 / math.sqrt(2.0) if u == 0 else 1.0
        for x in range(8):
            D[u, x] = au * 0.5 * math.cos((2 * x + 1) * u * math.pi / 16.0)
    return D


def _y_quant_table():
    t = np.array([[16, 11, 10, 16, 24, 40, 51, 61], [12, 12, 14, 19, 26, 58, 60, 55],
                  [14, 13, 16, 24, 40, 57, 69, 56], [14, 17, 22, 29, 51, 87, 80, 62],
                  [18, 22, 37, 56, 68, 109, 103, 77], [24, 35, 55, 64, 81, 104, 113, 92],
                  [49, 64, 78, 87, 103, 121, 120, 101], [72, 92, 95, 98, 112, 100, 103, 99]],
                 dtype=np.float64).T
    return t


def _c_quant_table():
    t = np.full((8, 8), 99, dtype=np.float64)
    t[:4, :4] = np.array([[17, 18, 24, 47], [18, 21, 26, 66], [24, 26, 56, 99],
                          [47, 66, 99, 99]], dtype=np.float64).T
    return t


_QOFF = 1536.0  # fp16 integer-grid rounding offset (ulp == 1 in [1024, 2048))

# scheduling knobs (tuned via TimelineSim sweep)
_CFG = dict(
    load_split={0: 2, 1: 2, 2: 2},  # plane -> number of input-DMA chunks
    head_pin=False,           # plane-0 h0 quant at [512] grain, pinned engines
    prefetch=3,               # planes loaded ahead
    tail_half=3,              # last-N planes ship output in halves
    xbufs=4,                  # x tile ring depth
    bal_act0=0.0,             # initial Act balancer offset
    b_first=False,            # zipper order: drain (B) before fill (A)
    late_load=False,          # emit prefetch load after the step, not before
    warm_mms=4,               # back-to-back dummy matmuls bridging PE idle
)


def _host_constants():
    D = _dct8_f64()
    M64 = np.kron(D, D)                       # [64,64] 2D DCT, orthonormal
    M2 = np.kron(np.eye(2), M64)              # [128,128] two blocks/partition

    # quant divisors along the partition axis: p%64 = 8u+v
    QT = np.stack([_y_quant_table(), _c_quant_table()])  # [2,8,8] (Y, C)
    p = np.arange(128)
    u, v = (p % 64) // 8, p % 8
    qvec = QT[:, u, v]                        # [2,128] divisor per partition

    mmF = np.asarray((M2 / 8.0).T, dtype=np.float16)        # fwd stationary
    mmI = [np.asarray(qvec[c][:, None] * M2 / 8.0, dtype=np.float16)
           for c in range(2)]                 # lhsT_i[k,p] = Q[k]*M2[k,p]/8

    # qs: quant scale per partition/channel-class; bias cancels the +_QOFF
    # offset after it flows through the (fp16-rounded) inverse stationary.
    qs = np.zeros((128, 2), dtype=np.float32)
    bias = np.zeros((128, 2), dtype=np.float32)
    for c in range(2):
        qs[:, c] = (8.0 / qvec[c]).astype(np.float32)
        colsum = mmI[c].astype(np.float64).sum(axis=0)      # [128]
        bias[:, c] = (128.0 - 8.0 * _QOFF * colsum).astype(np.float32)

    cmm = np.concatenate([mmF, mmI[0], mmI[1]], axis=1)     # [128, 384] f16
    csc = np.concatenate([qs, bias], axis=1)                # [128, 4] f32
    return dict(cmm=cmm, csc=csc)


def _build_program(cfg=None):
    import sys
    if "/opt/trn_rl_repo" not in sys.path:
        sys.path.insert(0, "/opt/trn_rl_repo")
    from contextlib import ExitStack
    import concourse.bacc as bacc
    import concourse.tile as tile
    from concourse import mybir
    from concourse.alu_op_type import AluOpType

    cfg = dict(_CFG, **(cfg or {}))

    F32 = mybir.dt.float32
    F16 = mybir.dt.float16
    U8 = mybir.dt.uint8

    nc = bacc.Bacc("TRN2", target_bir_lowering=False, debug=False,
                   num_devices=_N_CORES)

    # x: block-flattened YCC255-128 fp16 (host-prepped)
    x = nc.declare_dram_parameter("x", [_BPC, 3, 128, 2048], F16,
                                  isOutput=False)
    cmm = nc.declare_dram_parameter("cmm", [128, 384], F16, isOutput=False)
    csc = nc.declare_dram_parameter("csc", [128, 4], F32, isOutput=False)
    # out: YCC255 pixels, uint8, same block-flattened layout
    out = nc.declare_dram_parameter("out", [_BPC, 3, 128, 2048], U8,
                                    isOutput=True)

    U = 3 * _BPC  # 12 planes per core

    with tile.TileContext(nc) as tc, ExitStack() as ctx:
        cpool = ctx.enter_context(tc.tile_pool(name="consts", bufs=1))
        xpool = ctx.enter_context(tc.tile_pool(name="xp", bufs=cfg["xbufs"]))
        rqpool = ctx.enter_context(tc.tile_pool(name="rqp", bufs=3))
        opool = ctx.enter_context(tc.tile_pool(name="op", bufs=3))
        psF = ctx.enter_context(tc.tile_pool(name="psF", bufs=2, space="PSUM"))
        psI = ctx.enter_context(tc.tile_pool(name="psI", bufs=2, space="PSUM"))

        # Warm-up: a no-dependency matmul pins pe_busy_start at ~0 so real
        # matmuls start at full clock; a dummy activation pulls the Act
        # function-table load off the critical path.
        wz = cpool.tile([128, 512], F16, tag="warm")
        nc.vector.memset(wz[:], 0.0)
        wp = psF.tile([128, 2, 512], F32, tag="pf")
        for _ in range(cfg["warm_mms"]):
            nc.tensor.matmul(wp[:16, 0, :], wz[:, 0:16], wz[:],
                             start=True, stop=True)
        wa = cpool.tile([128, 16], F16, tag="warm2")
        nc.scalar.activation(wa[:], wz[:, 0:16],
                             mybir.ActivationFunctionType.Copy, scale=1.0)

        # consts on the Act HWDGE queue so they decode concurrently with the
        # first x loads on the SP queue
        t16 = cpool.tile([128, 384], F16, tag="c_f16")
        nc.scalar.dma_start(t16[:], cmm[:])
        t32 = cpool.tile([128, 4], F32, tag="c_f32")
        nc.scalar.dma_start(t32[:], csc[:])
        mmF = t16[:, 0:128]
        mmI = (t16[:, 128:256], t16[:, 256:384])  # Y, C stationaries

        st = {}
        # greedy Act/DVE balancer for the pointwise ops
        bal = {"act": cfg["bal_act0"], "dve": 0.0}

        def pick_engine(cost_act=1038.0, cost_dve=1192.0):
            if bal["act"] + cost_act <= bal["dve"] + cost_dve:
                bal["act"] += cost_act
                return "act"
            bal["dve"] += cost_dve
            return "dve"

        def emit_quant(dst, srcap, qs, eng):
            if eng == "act":
                nc.scalar.activation(dst, srcap,
                                     mybir.ActivationFunctionType.Copy,
                                     bias=_QOFF, scale=qs)
            else:
                nc.vector.tensor_scalar(dst, srcap, qs, _QOFF,
                                        op0=AluOpType.mult, op1=AluOpType.add)

        def emit_evict(dst, srcap, bias, eng):
            if eng == "act":
                nc.scalar.activation(dst, srcap,
                                     mybir.ActivationFunctionType.Identity,
                                     bias=bias, scale=8.0)
            else:
                nc.vector.tensor_scalar(dst, srcap, 8.0, bias,
                                        op0=AluOpType.mult, op1=AluOpType.add)

        def load_plane(u):
            img, ci = divmod(u, 3)
            t = xpool.tile([128, 2048], F16, tag="x")
            split = cfg["load_split"].get(u, 1)
            step = 2048 // split
            for s in range(split):
                nc.sync.dma_start(t[:, s * step:(s + 1) * step],
                                  x[img, ci][:, s * step:(s + 1) * step])
            st[u] = {"xt": t}

        def phaseA(u):
            # forward DCT + quantize (fp16 integer-grid RNE via +_QOFF)
            img, ci = divmod(u, 3)
            cc = 0 if ci == 0 else 1
            qs = t32[:, cc:cc + 1]
            xt = st[u]["xt"]
            rq = rqpool.tile([128, 2048], F16, tag="rq")
            st[u]["rq"] = rq
            for h in range(2):
                pf = psF.tile([128, 2, 512], F32, tag="pf")
                fine = (u == 0 and h == 0 and cfg["head_pin"])
                for j in range(2):
                    k = 2 * h + j
                    nc.tensor.matmul(pf[:, j, :], mmF,
                                     xt[:, k * 512:(k + 1) * 512],
                                     start=True, stop=True)
                    if fine:
                        eng = "act" if j == 0 else "dve"
                        bal[eng] += 612.0 if eng == "act" else 658.0
                        emit_quant(rq[:, k * 512:(k + 1) * 512], pf[:, j, :],
                                   qs, eng)
                if not fine:
                    emit_quant(rq[:, h * 1024:(h + 1) * 1024], pf[:, :, :],
                               qs, pick_engine())
                yield

        def phaseB(u):
            # inverse DCT (dequant folded) + biased uint8 eviction
            img, ci = divmod(u, 3)
            cc = 0 if ci == 0 else 1
            bias = t32[:, 2 + cc:3 + cc]
            rq = st[u]["rq"]
            ot = opool.tile([128, 2048], U8, tag="ot")
            for h in range(2):
                pi = psI.tile([128, 2, 512], F32, tag="pi")
                for j in range(2):
                    k = 2 * h + j
                    nc.tensor.matmul(pi[:, j, :], mmI[cc],
                                     rq[:, k * 512:(k + 1) * 512],
                                     start=True, stop=True)
                emit_evict(ot[:, h * 1024:(h + 1) * 1024], pi[:, :, :], bias,
                           pick_engine())
                if u >= U - cfg["tail_half"]:
                    # tail planes: ship each half as soon as it is evicted
                    nc.sync.dma_start(
                        out[img, ci][:, h * 1024:(h + 1) * 1024],
                        ot[:, h * 1024:(h + 1) * 1024])
                yield
            if u < U - cfg["tail_half"]:
                nc.sync.dma_start(out[img, ci], ot[:])
            del st[u]["rq"], st[u]["xt"]

        # software pipeline: A(u) runs alongside B(u-1); loads prefetched
        PF = cfg["prefetch"]
        for uu in range(min(PF, U)):
            load_plane(uu)
        prev_b = None
        for u in range(U):
            if u + PF < U and not cfg["late_load"]:
                load_plane(u + PF)
            a = phaseA(u)
            if cfg["b_first"]:
                gens = [g for g in (prev_b, a) if g is not None]
            else:
                gens = [g for g in (a, prev_b) if g is not None]
            while gens:
                for g in list(gens):
                    try:
                        next(g)
                    except StopIteration:
                        gens.remove(g)
            prev_b = phaseB(u)
            if u + PF < U and cfg["late_load"]:
                load_plane(u + PF)
        for _ in prev_b:
            pass

    nc.compile()
    return nc, _host_constants()


def _get_program():
    if "nc" not in _state:
        _state["nc"] = _build_program()
    return _state["nc"]


def _host_forward(image):
    """clip + RGB->YCbCr(255, -128) f32, then block-flatten to fp16.

    Layout: partition p = 64*s + 8*y + xx (s = block-row parity, y/xx =
    row/col within the 8x8 block), free j = (block_row//2)*64 + block_col.
    """
    x = np.clip(image.astype(np.float32, copy=False), 0.0, 1.0)
    r, g, b = x[:, 0], x[:, 1], x[:, 2]
    y = np.float32(0.299) * r + np.float32(0.587) * g + np.float32(0.114) * b
    cb = (b - y) * np.float32(0.564) + np.float32(0.5)
    cr = (r - y) * np.float32(0.713) + np.float32(0.5)
    ycc = np.stack([y, cb, cr], axis=1)
    v = ycc * np.float32(255.0) - np.float32(128.0)
    B = v.shape[0]
    t = v.reshape(B, 3, 32, 2, 8, 64, 8)          # [B,3,brh,s,y,bc,xx]
    x64 = t.transpose(0, 1, 3, 4, 6, 2, 5)        # [B,3,s,y,xx,brh,bc]
    return np.ascontiguousarray(x64.reshape(B, 3, 128, 2048),
                                dtype=np.float16)


def _host_inverse(y64u8):
    """y64u8: [B,3,128,2048] uint8 YCC255 block-flattened -> f32 RGB."""
    B = y64u8.shape[0]
    w = y64u8.reshape(B, 3, 2, 8, 8, 32, 64)      # [B,3,s,y,xx,brh,bc]
    v = w.transpose(0, 1, 5, 2, 3, 6, 4)          # [B,3,brh,s,y,bc,xx]
    px = v.reshape(B, 3, _H, _W).astype(np.float32) / np.float32(255.0)
    yy = px[:, 0]
    cb = px[:, 1] - np.float32(0.5)
    cr = px[:, 2] - np.float32(0.5)
    r = yy + np.float32(1.403) * cr
    g = yy - np.float32(0.714) * cr - np.float32(0.344) * cb
    b = yy + np.float32(1.773) * cb
    rgb = np.stack([r, g, b], axis=1)
    return np.clip(rgb, 0.0, 1.0).astype(np.float32)


def kernel(image: np.ndarray) -> np.ndarray:
    import sys
    if "/opt/trn_rl_repo" not in sys.path:
        sys.path.insert(0, "/opt/trn_rl_repo")
    from concourse.bass_utils import run_bass_kernel_spmd

    image = np.asarray(image)
    assert image.shape == (_B, 3, _H, _W), image.shape
    nc, consts = _get_program()

    x64 = _host_forward(image)                    # [32,3,128,2048] f16
    x64 = x64.reshape(_B, 3, 128, 2048)

    in_maps = []
    for c in range(_N_CORES):
        sl = slice(c * _BPC, (c + 1) * _BPC)
        m = dict(x=x64[sl])
        m.update(consts)
        in_maps.append(m)

    res = run_bass_kernel_spmd(nc, in_maps, core_ids=list(range(_N_CORES)))
    _state["exec_time_ns"] = getattr(res, "exec_time_ns", None)
    outs = [res.results[c]["out"] for c in range(_N_CORES)]
    yfull = np.concatenate(outs, axis=0)
    return _host_inverse(yfull)


if __name__ == "__main__":
    rng = np.random.default_rng(0)
    img = rng.uniform(size=(_B, 3, _H, _W)).astype(np.float32)
    o = kernel(img)
    print(o.shape, o.dtype, float(o.min()), float(o.max()))
